# revision 39
# baseline (speedup 1.0000x reference)
"""Trainium2 Bass kernel for nn_Attention (2-batch, 16-head, n=2048, d=64 causal
attention with LayerNorm-projected l2-normalized q/k, relative position bias,
and output projection), SPMD across 8 NeuronCores.

Single-launch design: each core tensor-parallels 2 of the 16 heads (both
batches) AND computes its partial of the output projection (its 128 rows of
Wo); the host sums the 8 partial outputs (bf16) into the full f32 result.

Key structure (v2, vs the two-launch baseline):
- rel_pos_bias is EXPONENTIATED on the host (EB = exp(bias^T), causal-masked
  to exact 0).  E = exp(sim)*EB via cheap bf16 DVE/gpsimd multiplies -- no
  f32 bias adds, no PE identity-injection matmuls.
- k-side l2norm rides the Exp activation's per-partition scale operand:
  rn_k[j] columns come from tiny transposed matmuls (sq_k-chunk stationary,
  ones moving -> [128 tok, 2 head] in PSUM) + one packed rsqrt on [128,4,2].
  Only the q-side needs the f32r broadcast matmul (rn_q is per-column).
- q-side combined scale s_e = q_scale*k_scale/8 folded into the qhat scaling.
- LN rstd applied to PE-transposed v rows by gpsimd tensor_scalar.
- Output projection per (b, chunk): 1/S row + f32r ones-broadcast matmul +
  DVE muls -> an [128, 512] bf16, then 8 matmuls against the core's 128 Wo
  rows, evacuated bf16 and DMA'd out.  normwo(ic, b0) is pulled as filler
  into phase2(ic, b1); normwo(ic, b1) into phase2(ic+1, b0) -- this also
  recycles the 2-deep avs PSUM ring without deadlock.
- Schedule: batch-interleaved projections; stats/phase1 of the next chunk
  pulled into phase2 tile groups as PE filler so the PE stream stays dense
  (HAM clock-gate stays at 2.4 GHz).
"""

import itertools
import numpy as np

HEADS = 16
DH = 64
B = 2
N = 2048
DIM = 1024
EH = 128          # per-core slice of the inner dim (2 heads x 64)
NCORES = 8
IC = 512          # i-chunk width
NIC = N // IC     # 4 i-chunks
JT = 128          # j-tile width
NJT = N // JT     # 16 j-tiles
NCT = DIM // 128  # 8 contraction tiles
LN_EPS = 1e-5
NEG = -1e30
N_WARM = 28


def _eb_mode(jt):
    """Per-j-tile bias application: 0 = PE identity-injects the RAW masked
    bias into the sim PSUM pre-exp (host stores raw bias rows for these
    tiles); 1 = DVE multiplies exp(bias) post-exp; 2 = gpsimd multiplies.
    k-side l2norm is folded into khat (sr matmul) so injection is exact."""
    return 1 if jt % 2 == 0 else 2


_cache = {}

# rsqrt-approx custom DVE ops: quadratic minimax seed on [0.22, 3.2] followed
# by two Newton-Raphson steps (~1.5e-3 max rel err on the clamped domain).
_RSQRT_C = (2.07776662, -1.18449153, 0.22938856)
_RSQRT_OPS = {}


def _register_rsqrt_ops():
    if _RSQRT_OPS:
        return _RSQRT_OPS
    from concourse import dve_ops
    from concourse.dve_spec import Spec, Src0, Src1, C0, C1, C2, lower, _has_src1
    from concourse.dve_uop import DveOpSpec

    def mk(name, body, ref):
        if name in dve_ops._SUB_OPCODE_FOR_NAME:
            for op in dve_ops.OPS:
                if op.name == name:
                    return op
        row = dve_ops._CUSTOM_DVE_ROW_BASE + len(dve_ops.OPS)
        spec = Spec(body=body, reference=ref)
        shas = {}
        for ver in ("v3", "v4"):
            uops = lower(spec, ver=ver)
            shas[ver] = DveOpSpec(name=name, opcode=row, uops=uops,
                                  rd1_en=_has_src1(spec)).sha(ver)
        op = dve_ops.DveOp(name, spec, subdim=False, uops_sha=shas)
        dve_ops.OPS.append(op)
        dve_ops.CUSTOM_DVE_SPECS[name] = spec
        dve_ops._SUB_OPCODE_FOR_NAME[name] = row
        return op

    seed = mk(
        "RSQRT_SEED_QUAD_ANT",
        (C0 + Src0 * C1) + (Src0 * Src0) * C2,
        lambda in0, in1, c0, c1, c2: (c0 + in0 * c1) + (in0 * in0) * c2,
    )
    nr = mk(
        "RSQRT_NR_ANT",
        Src1 * (C0 - (Src0 * (Src1 * Src1)) * C1),
        lambda in0, in1, c0, c1, c2: in1 * (c0 - (in0 * (in1 * in1)) * c1),
    )
    _RSQRT_OPS["seed"] = seed
    _RSQRT_OPS["nr"] = nr
    return _RSQRT_OPS


def _emit_rsqrt(nc, vec, out, xc, tmp_pool, tag):
    """out = x^-1/2 for xc pre-clamped to [0.22, 3.2]; out/xc f32."""
    ops = _register_rsqrt_ops()
    c0, c1, c2 = _RSQRT_C
    t1 = tmp_pool.tile(list(xc.shape), xc.dtype, tag=f"{tag}a", name=f"{tag}1")
    vec._custom_dve(ops["seed"], out=t1, in0=xc, s0=c0, s1=c1, imm2=c2)
    t2 = tmp_pool.tile(list(xc.shape), xc.dtype, tag=f"{tag}b", name=f"{tag}2")
    vec._custom_dve(ops["nr"], out=t2, in0=xc, in1=t1, s0=1.5, s1=0.5)
    vec._custom_dve(ops["nr"], out=out, in0=xc, in1=t2, s0=1.5, s1=0.5)


class _Chain:
    """Mutable filler chain with prepend support."""

    def __init__(self, *its):
        self.it = itertools.chain(*its)

    def prepend(self, it):
        self.it = itertools.chain(it, self.it)

    def pull(self, k):
        for _ in range(k):
            if next(self.it, None) is None:
                return False
        return True

    def drain(self):
        for _ in self.it:
            pass


def _build_launch():
    import concourse.bass as bass
    import concourse.tile as tile
    from concourse import bacc, mybir
    from concourse.masks import make_identity

    F32 = mybir.dt.float32
    F32R = mybir.dt.float32r
    BF16 = mybir.dt.bfloat16
    AF = mybir.ActivationFunctionType
    nc = bacc.Bacc(None)
    xT_d = nc.declare_dram_parameter("xT", [B, NIC, 2, 512, IC], BF16,
                                     isOutput=False)
    xn_d = nc.declare_dram_parameter("x_nat", [B, NIC, 4, 128, 2, 512], BF16,
                                     isOutput=False)
    eb_d = nc.declare_dram_parameter("ebT", [2, N, N], BF16, isOutput=False)
    wq_d = nc.declare_dram_parameter("wq", [DIM, EH], BF16, isOutput=False)
    wk_d = nc.declare_dram_parameter("wk", [DIM, EH], BF16, isOutput=False)
    wv_d = nc.declare_dram_parameter("wv", [DIM, EH], BF16, isOutput=False)
    wo_d = nc.declare_dram_parameter("wo", [EH, DIM], BF16, isOutput=False)
    qs2_d = nc.declare_dram_parameter("qs2", [EH], F32, isOutput=False)
    kb_d = nc.declare_dram_parameter("kb", [B, N], F32, isOutput=False)
    out_d = nc.declare_dram_parameter("out_part", [B, N, DIM], BF16,
                                      isOutput=True)

    with tile.TileContext(nc) as tc:
        import contextlib
        with contextlib.ExitStack() as ctx:
            pers = ctx.enter_context(tc.tile_pool(name="pers", bufs=1))
            # SBUF pools
            xrp = ctx.enter_context(tc.tile_pool(name="xrp", bufs=2))
            xnp = ctx.enter_context(tc.tile_pool(name="xnp", bufs=2))
            eb_pool = ctx.enter_context(tc.tile_pool(name="eb_pool", bufs=2))
            rowp = ctx.enter_context(tc.tile_pool(name="rowp", bufs=1))
            colp = ctx.enter_context(tc.tile_pool(name="colp", bufs=2))
            sqp = ctx.enter_context(tc.tile_pool(name="sqp", bufs=2))
            srp = ctx.enter_context(tc.tile_pool(name="srp", bufs=1))
            ep = ctx.enter_context(tc.tile_pool(name="ep", bufs=3))
            emp = ctx.enter_context(tc.tile_pool(name="emp", bufs=5))
            anp = ctx.enter_context(tc.tile_pool(name="anp", bufs=2))
            obp = ctx.enter_context(tc.tile_pool(name="obp", bufs=2))
            # PSUM pools: tl(4) + sp(2) + av(2) = 8 banks
            tl_ps = ctx.enter_context(tc.tile_pool(name="tl_ps", bufs=4, space="PSUM"))
            sp_ps = ctx.enter_context(tc.tile_pool(name="sp_ps", bufs=2, space="PSUM"))
            av_ps = ctx.enter_context(tc.tile_pool(name="av_ps", bufs=2, space="PSUM"))

            # ---------- constants ----------
            onescol_f = pers.tile([128, 1], F32, tag="onescol_f")
            nc.vector.memset(onescol_f, 1.0)
            onescol_bf = pers.tile([128, 1], BF16, tag="onescol_bf")
            nc.vector.tensor_copy(out=onescol_bf, in_=onescol_f)
            warm_row = pers.tile([1, 512], BF16, tag="warm_row")
            nc.vector.memset(warm_row, 1.0)
            # q-side block-ones with 1/64 folded (ssq = |q|^2/64 per head)
            o2q_f = pers.tile([128, 2], F32, tag="o2q_f")
            nc.vector.memset(o2q_f, 0.0)
            nc.vector.memset(o2q_f[0:64, 0:1], 1.0 / 64.0)
            nc.vector.memset(o2q_f[64:128, 1:2], 1.0 / 64.0)
            ones2blk_q = pers.tile([128, 2], BF16, tag="ones2blk_q")
            nc.vector.tensor_copy(out=ones2blk_q, in_=o2q_f)
            ident = pers.tile([128, 128], F32, tag="ident")
            make_identity(nc, ident)
            ident_bf = pers.tile([128, 128], BF16, tag="ident_bf")
            nc.vector.tensor_copy(out=ident_bf, in_=ident)

            # ---- PE warm-up: dummy matmuls while the first DMAs stream ----
            warm_ps = av_ps.tile([1, IC], F32, tag="av")
            for _ in range(N_WARM):
                nc.tensor.matmul(warm_ps, onescol_bf[0:1, :], warm_row,
                                 start=True, stop=True)

            # ---------- weights (gamma-folded + centered on host) ----------
            wps = {}
            for nm, wd in (("q", wq_d),):
                wp = pers.tile([128, NCT, EH], BF16, tag=f"w{nm}p")
                nc.sync.dma_start(out=wp, in_=wd.ap().rearrange("(t p) e -> p t e", p=128))
                wps[nm] = wp

            # scale rows -> block-diag [2, 128] (qsb[h, e] = s[e] iff head(e)==h)
            # where s = q_scale * k_scale / 8 (prepped on host)
            qsb_f = pers.tile([2, 128], F32, tag="qsb_f")
            nc.vector.memset(qsb_f, 0.0)
            nc.sync.dma_start(out=qsb_f[0:1, 0:64], in_=qs2_d.ap()[0:64].unsqueeze(0))
            nc.sync.dma_start(out=qsb_f[1:2, 64:128], in_=qs2_d.ap()[64:128].unsqueeze(0))
            qs2blk = pers.tile([2, 128], F32R, tag="qs2blk")
            nc.vector.tensor_copy(out=qs2blk, in_=qsb_f)

            kbT = pers.tile([128, B, NJT], F32, tag="kbT")
            nc.sync.dma_start(out=kbT, in_=kb_d.ap().rearrange("b (t p) -> p b t", p=128))

            wo_sb = pers.tile([128, DIM], BF16, tag="wo_sb")
            nc.sync.dma_start(out=wo_sb, in_=wo_d.ap())

            # ---------- persistent per-batch products ----------
            qhat = [pers.tile([128, N], BF16, tag=f"qhat{b}", name=f"qhat{b}") for b in range(B)]
            khat = [pers.tile([128, N], BF16, tag=f"khat{b}", name=f"khat{b}") for b in range(B)]
            v_all = [pers.tile([128, NJT, 130], BF16, tag=f"vall{b}", name=f"vall{b}") for b in range(B)]
            rnk = [pers.tile([128, NIC, 4, 2], F32, tag=f"rnk{b}", name=f"rnk{b}")
                   for b in range(B)]
            for b in range(B):
                # softmax-denominator columns (65h+64) are constant 1.0
                for jt in range(NJT):
                    nc.vector.memset(v_all[b][:, jt, 64:65], 1.0)
                    nc.vector.memset(v_all[b][:, jt, 129:130], 1.0)

            def emit_xr_chunk(icn):
                """xT chunk tiles [128, NCT, IC] per b, from a 2-buf ring."""
                xrs = []
                for b in range(B):
                    xr = xrp.tile([128, NCT, IC], BF16, tag=f"xr{b}",
                                  name=f"xr{b}_{icn}")
                    for half in range(2):
                        hs = slice(half * (NCT // 2), (half + 1) * (NCT // 2))
                        nc.sync.dma_start(
                            out=xr[:, hs, :],
                            in_=xT_d.ap()[b, icn, half].rearrange(
                                "(t p) n -> p t n", p=128))
                    xrs.append(xr)
                return xrs

            def emit_xna_chunk(ic):
                xnas = []
                for b in range(B):
                    xna = xnp.tile([128, 4, 2, 512], BF16, tag=f"xna{b}",
                                   name=f"xna{b}_{ic}")
                    nc.scalar.dma_start(
                        out=xna, in_=xn_d.ap()[b, ic].rearrange("t p g f -> p t g f"))
                    xnas.append(xna)
                return xnas

            def eb_prefetch(ic):
                """EB chunk tile [128, NJT, 2, IC] (jt-major, head-interleaved)."""
                isl = slice(ic * IC, (ic + 1) * IC)
                jmax = 4 * (ic + 1)
                ebt = eb_pool.tile([128, NJT, 2, IC], BF16, tag="eb",
                                   name=f"eb{ic}")
                for h in range(2):
                    nc.gpsimd.dma_start(
                        out=ebt[:, 0:jmax, h, :],
                        in_=eb_d.ap()[h, 0:jmax * 128, isl].rearrange(
                            "(t p) i -> p t i", p=128))
                return ebt

            def stats_units(ic, xnas, rstd_out):
                """LN variance per token (DVE bn_stats) -> rstd columns."""
                for b in range(B):
                    xna = xnas[b]
                    bag_all = colp.tile([128, 4, 2], F32, tag="bag")
                    for k in range(IC // 128):
                        bst = colp.tile([128, 2, 6], F32, tag="bst")
                        nc.vector.bn_stats(out=bst[:, 0, :], in_=xna[:, k, 0, :])
                        nc.vector.bn_stats(out=bst[:, 1, :], in_=xna[:, k, 1, :])
                        nc.vector.bn_aggr(out=bag_all[:, k, :], in_=bst)
                        yield
                    rstd_all = colp.tile([128, 4], F32, tag="rstdc")
                    _emit_rsqrt(nc, nc.vector, rstd_all, bag_all[:, :, 1], colp, "rsc")
                    rstd_out[b] = rstd_all
                    yield

            def phase1_units(ic, xrs, rstd_cols):
                """Chunk-ic projections + l2norm + v transpose, emitted
                batch-sequentially so phase2(ic, b0) can start as soon as
                batch 0 is done."""
                isl = slice(ic * IC, (ic + 1) * IC)
                for b in range(B):
                    xr = xrs[b]
                    # q projection + l2norm chain
                    p = tl_ps.tile([128, IC], F32, tag="tl", name=f"pq{b}")
                    for ct in range(NCT):
                        nc.tensor.matmul(p, wps["q"][:, ct, :], xr[:, ct, :],
                                         start=(ct == 0), stop=(ct == NCT - 1))
                    yield
                    sq = sqp.tile([128, IC], BF16, tag="sq", name=f"sqq{b}")
                    nc.scalar.activation(out=sq, in_=p, func=AF.Square)
                    ssq = sp_ps.tile([2, IC], F32, tag="sp", name=f"ssq{b}")
                    nc.tensor.matmul(ssq, ones2blk_q, sq, start=True, stop=True)
                    rnr = rowp.tile([2, IC], F32R, tag="rowtmp")
                    _emit_rsqrt(nc, nc.vector, rnr, ssq, rowp, "rsr")
                    sr = sp_ps.tile([128, IC], F32, tag="sp", name=f"sr{b}")
                    nc.tensor.matmul(sr, qs2blk, rnr, start=True, stop=True)
                    srs = srp.tile([128, IC], F32, tag="srs")
                    nc.vector.tensor_copy(out=srs, in_=sr)
                    nc.vector.tensor_mul(qhat[b][:, isl], p, srs)
                    yield
                    # k projection + rn_k columns
                    p = tl_ps.tile([128, IC], F32, tag="tl", name=f"pk{b}")
                    for ct in range(NCT):
                        nc.tensor.matmul(p, wps["k"][:, ct, :], xr[:, ct, :],
                                         start=(ct == 0), stop=(ct == NCT - 1))
                    yield
                    sq = sqp.tile([128, IC], BF16, tag="sq", name=f"sqk{b}")
                    nc.scalar.activation(out=sq, in_=p, func=AF.Square)
                    nc.scalar.copy(out=khat[b][:, isl], in_=p)
                    rncol = sp_ps.tile([128, 4, 2], F32, tag="sp", name=f"rnc{b}")
                    for jb in range(4):
                        nc.tensor.matmul(rncol[:, jb, :],
                                         sq[:, jb * 128:(jb + 1) * 128],
                                         ones2blk_q, start=True, stop=True)
                    _emit_rsqrt(nc, nc.vector, rnk[b][:, ic], rncol, colp, "rsk")
                    yield
                    # v projection + transpose + rstd row scale
                    p = tl_ps.tile([128, IC], F32, tag="tl", name=f"pv{b}")
                    for ct in range(NCT):
                        nc.tensor.matmul(p, wps["v"][:, ct, :], xr[:, ct, :],
                                         start=(ct == 0), stop=(ct == NCT - 1))
                    yield
                    rstd_all = rstd_cols[b]
                    vsc = sqp.tile([128, IC], BF16, tag="vsc")
                    nc.scalar.copy(out=vsc, in_=p)
                    for k in range(IC // 128):
                        jt = ic * (IC // 128) + k
                        vt = sp_ps.tile([128, 128], BF16, tag="sp", name=f"vt{b}{k}")
                        nc.tensor.transpose(vt, vsc[:, k * 128:(k + 1) * 128], ident_bf)
                        nc.vector.tensor_scalar_mul(
                            out=v_all[b][:, jt, 0:64], in0=vt[:, 0:64],
                            scalar1=rstd_all[:, k:k + 1])
                        nc.vector.tensor_scalar_mul(
                            out=v_all[b][:, jt, 65:129], in0=vt[:, 64:128],
                            scalar1=rstd_all[:, k:k + 1])
                    yield

            def normwo_units(ic, b, avs):
                """Normalize chunk-ic attention outputs and run the Wo partial
                matmuls.  avs = [h0, h1] PSUM tiles [65, IC]."""
                # 1/S rows: f32 reciprocal -> bf16 row for the ones-broadcast
                rec = rowp.tile([1, 2, IC], F32, tag="rec")
                nc.vector.tensor_copy(out=rec[:, 0, :], in_=avs[0][64:65, :])
                nc.vector.tensor_copy(out=rec[:, 1, :], in_=avs[1][64:65, :])
                nc.vector.reciprocal_approx_fast(out=rec, in_=rec)
                recb = rowp.tile([1, 2, IC], BF16, tag="recb")
                nc.vector.tensor_copy(out=recb, in_=rec)
                recbs = [recb[:, 0, :], recb[:, 1, :]]
                yield
                bcb = sp_ps.tile([128, IC], F32, tag="sp", name=f"bcb{ic}{b}")
                nc.tensor.matmul(bcb[0:64, :], warm_row[:, 0:64], recbs[0],
                                 start=True, stop=True)
                nc.tensor.matmul(bcb[64:128, :], warm_row[:, 0:64], recbs[1],
                                 start=True, stop=True)
                bcs = srp.tile([128, IC], F32, tag="bcs")
                nc.vector.tensor_copy(out=bcs, in_=bcb)
                yield
                an = anp.tile([128, IC], BF16, tag="an")
                nc.vector.tensor_mul(an[0:64, :], avs[0][0:64, :], bcs[0:64, :])
                nc.vector.tensor_mul(an[64:128, :], avs[1][0:64, :], bcs[64:128, :])
                yield
                for it in range(4):
                    ob = obp.tile([128, DIM], BF16, tag="ob")
                    for oc in range(2):
                        op = sp_ps.tile([128, 512], F32, tag="sp",
                                        name=f"op{ic}{b}{it}{oc}")
                        nc.tensor.matmul(op, an[:, it * 128:(it + 1) * 128],
                                         wo_sb[:, oc * 512:(oc + 1) * 512],
                                         start=True, stop=True)
                        if oc == 0:
                            nc.vector.tensor_copy(out=ob[:, 0:512], in_=op)
                        else:
                            nc.scalar.copy(out=ob[:, 512:1024], in_=op)
                    nc.sync.dma_start(
                        out=out_d.ap()[b, ic * IC + it * 128:ic * IC + (it + 1) * 128, :],
                        in_=ob)
                    yield

            def phase2_chunk(ic, ebt, filler, pull_k):
                jmax = (IC // 128) * (ic + 1)
                avs_last = None
                for b in range(B):
                    avs = [av_ps.tile([65, IC], F32, tag="av", name=f"av{ic}{b}{h}")
                           for h in range(2)]
                    tiles = []
                    for jt in range(jmax):
                        diag_k = jt - (jmax - 4)
                        off = 128 * diag_k if diag_k > 0 else 0
                        tiles.append((jt, off))
                    # groups of 2 jt (= 4 (jt,h) tiles): sim mms back-to-back,
                    # exps, EB muls, pull filler, then the PREVIOUS group's
                    # av mms (their inputs are certainly ready).
                    groups = [tiles[i:i + 2] for i in range(0, len(tiles), 2)]
                    pend_group = []
                    for g in groups:
                        cur = []
                        for (jt, off) in g:
                            icj, jb = jt // 4, jt % 4
                            mode = _eb_mode(jt)
                            E = ep.tile([128, 2, IC], BF16, tag="E")
                            for h in range(2):
                                dsl = slice(64 * h, 64 * h + 64)
                                sp = tl_ps.tile([128, IC], F32, tag="tl", name="sim")
                                if mode == 0:
                                    # PE identity-injects the raw bias; NOTE:
                                    # the injected bias is NOT scaled by rn_k,
                                    # so the host pre-divides... see _eb_mode.
                                    nc.tensor.matmul(
                                        sp[:, off:], ident_bf,
                                        ebt[:, jt, h, off:],
                                        start=True, stop=False)
                                    nc.tensor.matmul(
                                        sp[:, off:], khat[b][dsl, jt * 128:(jt + 1) * 128],
                                        qhat[b][dsl, ic * IC + off:(ic + 1) * IC],
                                        start=False, stop=True)
                                else:
                                    nc.tensor.matmul(
                                        sp[:, off:], khat[b][dsl, jt * 128:(jt + 1) * 128],
                                        qhat[b][dsl, ic * IC + off:(ic + 1) * IC],
                                        start=True, stop=True)
                                nc.scalar.activation(
                                    out=E[:, h, off:], in_=sp[:, off:], func=AF.Exp,
                                    bias=kbT[:, b, jt:jt + 1],
                                    scale=rnk[b][:, icj, jb, h:h + 1])
                            if mode == 0:
                                cur.append((E, jt, off))
                            else:
                                Em = emp.tile([128, 2, IC], BF16, tag="Em")
                                eng = nc.gpsimd if mode == 2 else nc.vector
                                eng.tensor_mul(Em[:, :, off:], E[:, :, off:],
                                               ebt[:, jt, :, off:])
                                cur.append((Em, jt, off))
                        filler.pull(pull_k)
                        for (pEm, pjt, poff) in pend_group:
                            for h in range(2):
                                nc.tensor.matmul(
                                    avs[h][:, poff:],
                                    v_all[b][:, pjt, 65 * h:65 * h + 65],
                                    pEm[:, h, poff:], start=(pjt == 0),
                                    stop=(pjt == jmax - 1))
                        pend_group = cur
                    for (pEm, pjt, poff) in pend_group:
                        for h in range(2):
                            nc.tensor.matmul(
                                avs[h][:, poff:], v_all[b][:, pjt, 65 * h:65 * h + 65],
                                pEm[:, h, poff:], start=(pjt == 0),
                                stop=(pjt == jmax - 1))
                    if b == 0:
                        # b0's norm+Wo becomes the front of the filler so its
                        # avs PSUM bank frees before b1's av accumulation.
                        filler.prepend(normwo_units(ic, 0, avs))
                        filler.pull(3)
                    else:
                        avs_last = avs
                return avs_last

            # ---------------- schedule ----------------
            xrs0 = emit_xr_chunk(0)
            ebt = eb_prefetch(0)
            for nm, wd in (("k", wk_d), ("v", wv_d)):
                wp = pers.tile([128, NCT, EH], BF16, tag=f"w{nm}p")
                nc.sync.dma_start(out=wp, in_=wd.ap().rearrange("(t p) e -> p t e", p=128))
                wps[nm] = wp
            xnas0 = emit_xna_chunk(0)
            rstd0 = {}
            stats0 = stats_units(0, xnas0, rstd0)
            p1_0 = phase1_units(0, xrs0, rstd0)
            # batch-0 of phase1(0) (+ its stats before the v epilogue); batch 1
            # is pulled into phase2(0, b0) as filler
            for _ in range(5):
                next(p1_0, None)
            for _ in range(5):
                next(stats0, None)
            next(p1_0, None)
            for _ in stats0:
                pass
            for _ in p1_0:
                pass

            xrs_next = emit_xr_chunk(1)
            xnas_next = emit_xna_chunk(1)

            pend_b1 = None   # (ic, avs) of previous chunk's b1 awaiting norm+Wo
            for ic in range(NIC):
                nebt = eb_prefetch(ic + 1) if ic + 1 < NIC else None
                fillers = []
                if ic == 0:
                    fillers.append(stats0)
                    fillers.append(p1_0)
                if pend_b1 is not None:
                    fillers.append(normwo_units(pend_b1[0], 1, pend_b1[1]))
                nrstd = {}
                if ic + 1 < NIC:
                    fillers.append(stats_units(ic + 1, xnas_next, nrstd))
                    fillers.append(phase1_units(ic + 1, xrs_next, nrstd))
                filler = _Chain(*fillers)
                groups_per_b = max(1, (4 * (ic + 1)) // 2)
                pull_k = max(2, min(6, -(-36 // (2 * groups_per_b))))
                avs_b1 = phase2_chunk(ic, ebt, filler, pull_k)
                filler.drain()
                pend_b1 = (ic, avs_b1)
                if ic + 2 < NIC:
                    xrs_next = emit_xr_chunk(ic + 2)
                    xnas_next = emit_xna_chunk(ic + 2)
                ebt = nebt
            # tail: last chunk's b1 norm+Wo
            for _ in normwo_units(pend_b1[0], 1, pend_b1[1]):
                pass
    nc.compile()
    return nc


PROFILE = {"enabled": False, "a_ns": None, "b_ns": None}


def _install_profile_hook():
    """Register the axon NTFF profile hook (the image's antenv lacks
    axon_hooks, so run_bass_kernel_spmd(trace=True) would silently skip
    tracing).  Replicates trn_boot's ctypes recipe."""
    import sys, types, ctypes, contextlib

    if "antenv.axon_hooks" in sys.modules:
        return
    lib = ctypes.CDLL("/opt/axon/libaxon_pjrt.so")
    if not hasattr(lib, "axon_start_nrt_profile"):
        return
    lib.axon_start_nrt_profile.argtypes = [ctypes.POINTER(ctypes.c_int64), ctypes.c_size_t]
    lib.axon_start_nrt_profile.restype = ctypes.c_int64
    lib.axon_stop_nrt_profile.argtypes = [ctypes.c_char_p]
    lib.axon_stop_nrt_profile.restype = ctypes.c_int64

    @contextlib.contextmanager
    def _hook(output_dir, device_ids):
        import jax
        jax.devices()
        if device_ids:
            ids = (ctypes.c_int64 * len(device_ids))(*device_ids)
            rc = lib.axon_start_nrt_profile(ids, len(device_ids))
        else:
            rc = lib.axon_start_nrt_profile(None, 0)
        if rc != 0:
            raise RuntimeError(f"axon_start_nrt_profile rc={rc}")
        try:
            yield
        finally:
            n = lib.axon_stop_nrt_profile(str(output_dir).encode())
            print(f"profile: {n} file(s) written to {output_dir}")

    mod = types.ModuleType("antenv.axon_hooks")
    mod.get_axon_ntff_profile_hook = lambda: _hook
    mod.set_axon_ntff_profile_hook = lambda h: None
    sys.modules["antenv.axon_hooks"] = mod

    from concourse import bass_utils
    bass_utils.upload_artifacts = lambda tmpdir: ""


def kernel(x, gamma, Wq, Wkv, q_scale, k_scale, Wo, rel_pos_bias, mask):
    from concourse.bass_utils import run_bass_kernel_spmd
    import ml_dtypes

    BF = ml_dtypes.bfloat16
    x = np.ascontiguousarray(np.asarray(x, dtype=np.float32))
    gamma = np.asarray(gamma, dtype=np.float32)
    Wq = np.asarray(Wq, dtype=np.float32)
    Wkv = np.asarray(Wkv, dtype=np.float32)
    q_scale = np.asarray(q_scale, dtype=np.float32)
    k_scale = np.asarray(k_scale, dtype=np.float32)
    Wo = np.ascontiguousarray(np.asarray(Wo, dtype=np.float32))
    rel_pos_bias = np.asarray(rel_pos_bias, dtype=np.float32)
    mask = np.asarray(mask)

    if PROFILE["enabled"]:
        _install_profile_hook()
    if "a" not in _cache:
        _cache["a"] = _build_launch()

    # xT layout: [B, NIC, half, 512(dim-part), IC]
    xT = np.ascontiguousarray(x.transpose(0, 2, 1)).astype(BF)   # [B, DIM, N]
    xT = np.ascontiguousarray(
        xT.reshape(B, 2, 512, NIC, IC).transpose(0, 3, 1, 2, 4))
    x_nat = np.ascontiguousarray(x.reshape(B, NIC, 4, 128, 2, 512)).astype(BF)
    kb = np.where(mask, 0.0, NEG).astype(np.float32)
    # combined q-side scale: q_scale * k_scale / 8  (k-side l2norm rides the
    # Exp scale operand as rn_k = 8/|k|; (1/8)*8*8 = SCALE=8 total)
    qs2 = np.tile(q_scale * k_scale / 8.0, 2).astype(np.float32)

    # gamma-fold + column-center the projection weights (exact LN mean-sub)
    def prep_w(W):
        Wg = W * gamma[:, None]
        return (Wg - Wg.mean(axis=0, keepdims=True)).astype(BF)

    Wq_p = prep_w(Wq)
    Wk_p = prep_w(Wkv[:, :DIM])
    Wv_p = prep_w(Wkv[:, DIM:])

    # rel_pos bias: transpose to [h, j, i]; per j-tile either RAW masked bias
    # (PE-injected pre-exp) or exp(bias) masked to 0 (post-exp multiply)
    rpbT = np.ascontiguousarray(rel_pos_bias.transpose(0, 2, 1))
    tri = np.tril(np.ones((N, N), dtype=bool), -1)   # [j, i]: j > i masked
    EB = np.exp(rpbT, dtype=np.float32)
    EB[:, tri] = 0.0
    EB = EB.astype(BF)

    Wo_bf = Wo.astype(BF)
    in_maps = []
    for c in range(NCORES):
        es = slice(EH * c, EH * (c + 1))
        in_maps.append({
            "xT": xT,
            "x_nat": x_nat,
            "ebT": np.ascontiguousarray(EB[2 * c:2 * c + 2]),
            "wq": np.ascontiguousarray(Wq_p[:, es]),
            "wk": np.ascontiguousarray(Wk_p[:, es]),
            "wv": np.ascontiguousarray(Wv_p[:, es]),
            "wo": np.ascontiguousarray(Wo_bf[es, :]),
            "qs2": qs2, "kb": kb,
        })
    res = run_bass_kernel_spmd(_cache["a"], in_maps, list(range(NCORES)),
                               trace=PROFILE["enabled"])
    if PROFILE["enabled"]:
        PROFILE["a_ns"] = res.exec_time_ns
        PROFILE["b_ns"] = 0

    out = np.zeros((B, N, DIM), np.float32)
    for c in range(NCORES):
        out += res.results[c]["out_part"].astype(np.float32)
    return out


# revision 40
# speedup vs baseline: 1.0036x; 1.0036x over previous
"""Trainium2 Bass kernel for nn_Attention (2-batch, 16-head, n=2048, d=64 causal
attention with LayerNorm-projected l2-normalized q/k, relative position bias,
and output projection), SPMD across 8 NeuronCores.

Single-launch design (vs the two-launch baseline): each core tensor-parallels
2 of the 16 heads (both batches) AND computes its partial of the output
projection (its 128 rows of Wo); the host sums the 8 bf16 partial outputs
into the full f32 result.  This removes launch B entirely (its fixed
overhead, the Wo/attn DRAM round-trip, and its cold-clock matmuls).

Key structure:
- rel_pos_bias is EXPONENTIATED on the host (EB = exp(bias^T), causal-masked
  to exact 0).  E = exp(sim)*EB via bf16 multiplies split DVE/gpsimd per
  j-tile -- no f32 bias adds, no PE identity-injection matmuls.
- k-side l2norm rides the Exp activation's per-partition scale operand:
  rn_k[j] columns come from tiny transposed matmuls (sq_k chunk as the
  stationary, block-ones moving -> [128 tok, 2 head] in PSUM) + one packed
  rsqrt on [128,4,2].  Only the q-side needs the f32r broadcast matmul
  (rn_q is per-i-column).  Combined scale q_scale*k_scale/8 folds into qhat.
- LN mean-subtraction is exact via gamma-folded column-centered weights; the
  per-token LN rstd (DVE bn_stats on a natural-layout x copy, loaded on the
  ACT engine's DMA queue) cancels in q/k and scales the PE-transposed v rows.
- All rsqrts use a custom DVE op chain (quadratic seed + 2 Newton steps); the
  scalar engine only runs {Exp, Square, Copy} -- one activation-table set.
- Output projection per (b, chunk): denominator row -> SBUF -> batched
  reciprocal -> bf16 row; ones-broadcast matmul -> [128,512] PSUM; DVE muls
  -> an bf16; 8 matmuls against the core's 128 Wo rows; evacuations split
  DVE/ACT; bf16 partials DMA'd straight to their [B,N,DIM] positions.
- Schedule: per-batch phase1 (projections dense on PE); phase2 in groups of
  2 j-tiles (sims -> exps -> EB muls -> previous group's av matmuls) with
  stats/phase1 of chunk ic+1 and the norm+Wo of completed chunks pulled in
  as PE filler between groups; norm+Wo(ic, b0) is prepended into
  phase2(ic, b1) so the 2-deep avs PSUM ring recycles without deadlock.
- PSUM: tl(4: projections+sims) + sp(2: l2norm/transposes/broadcast/Wo) +
  av(2: attn accumulators) = 8 banks.  Warm-up matmuls cover the initial
  x-chunk DMA so the PE HAM clock-gate arms before the first projection.
"""

import itertools
import numpy as np

HEADS = 16
DH = 64
B = 2
N = 2048
DIM = 1024
EH = 128          # per-core slice of the inner dim (2 heads x 64)
NCORES = 8
IC = 512          # i-chunk width
NIC = N // IC     # 4 i-chunks
JT = 128          # j-tile width
NJT = N // JT     # 16 j-tiles
NCT = DIM // 128  # 8 contraction tiles
LN_EPS = 1e-5
NEG = -1e30
N_WARM = 28


def _eb_mode(jt):
    """Per-j-tile bias application: 0 = PE identity-injects the RAW masked
    bias into the sim PSUM pre-exp (host stores raw bias rows for these
    tiles); 1 = DVE multiplies exp(bias) post-exp; 2 = gpsimd multiplies.
    k-side l2norm is folded into khat (sr matmul) so injection is exact."""
    return 1 if jt % 2 == 0 else 2


_cache = {}

# rsqrt-approx custom DVE ops: quadratic minimax seed on [0.22, 3.2] followed
# by two Newton-Raphson steps (~1.5e-3 max rel err on the clamped domain).
_RSQRT_C = (2.07776662, -1.18449153, 0.22938856)
_RSQRT_OPS = {}


def _register_rsqrt_ops():
    if _RSQRT_OPS:
        return _RSQRT_OPS
    from concourse import dve_ops
    from concourse.dve_spec import Spec, Src0, Src1, C0, C1, C2, lower, _has_src1
    from concourse.dve_uop import DveOpSpec

    def mk(name, body, ref):
        if name in dve_ops._SUB_OPCODE_FOR_NAME:
            for op in dve_ops.OPS:
                if op.name == name:
                    return op
        row = dve_ops._CUSTOM_DVE_ROW_BASE + len(dve_ops.OPS)
        spec = Spec(body=body, reference=ref)
        shas = {}
        for ver in ("v3", "v4"):
            uops = lower(spec, ver=ver)
            shas[ver] = DveOpSpec(name=name, opcode=row, uops=uops,
                                  rd1_en=_has_src1(spec)).sha(ver)
        op = dve_ops.DveOp(name, spec, subdim=False, uops_sha=shas)
        dve_ops.OPS.append(op)
        dve_ops.CUSTOM_DVE_SPECS[name] = spec
        dve_ops._SUB_OPCODE_FOR_NAME[name] = row
        return op

    seed = mk(
        "RSQRT_SEED_QUAD_ANT",
        (C0 + Src0 * C1) + (Src0 * Src0) * C2,
        lambda in0, in1, c0, c1, c2: (c0 + in0 * c1) + (in0 * in0) * c2,
    )
    nr = mk(
        "RSQRT_NR_ANT",
        Src1 * (C0 - (Src0 * (Src1 * Src1)) * C1),
        lambda in0, in1, c0, c1, c2: in1 * (c0 - (in0 * (in1 * in1)) * c1),
    )
    _RSQRT_OPS["seed"] = seed
    _RSQRT_OPS["nr"] = nr
    return _RSQRT_OPS


def _emit_rsqrt(nc, vec, out, xc, tmp_pool, tag):
    """out = x^-1/2 for xc pre-clamped to [0.22, 3.2]; out/xc f32."""
    ops = _register_rsqrt_ops()
    c0, c1, c2 = _RSQRT_C
    t1 = tmp_pool.tile(list(xc.shape), xc.dtype, tag=f"{tag}a", name=f"{tag}1")
    vec._custom_dve(ops["seed"], out=t1, in0=xc, s0=c0, s1=c1, imm2=c2)
    t2 = tmp_pool.tile(list(xc.shape), xc.dtype, tag=f"{tag}b", name=f"{tag}2")
    vec._custom_dve(ops["nr"], out=t2, in0=xc, in1=t1, s0=1.5, s1=0.5)
    vec._custom_dve(ops["nr"], out=out, in0=xc, in1=t2, s0=1.5, s1=0.5)


class _Chain:
    """Mutable filler chain with prepend support."""

    def __init__(self, *its):
        self.it = itertools.chain(*its)

    def prepend(self, it):
        self.it = itertools.chain(it, self.it)

    def pull(self, k):
        for _ in range(k):
            if next(self.it, None) is None:
                return False
        return True

    def drain(self):
        for _ in self.it:
            pass


def _build_launch():
    import concourse.bass as bass
    import concourse.tile as tile
    from concourse import bacc, mybir
    from concourse.masks import make_identity

    F32 = mybir.dt.float32
    F32R = mybir.dt.float32r
    BF16 = mybir.dt.bfloat16
    AF = mybir.ActivationFunctionType
    nc = bacc.Bacc(None)
    xT_d = nc.declare_dram_parameter("xT", [B, NIC, 2, 512, IC], BF16,
                                     isOutput=False)
    xn_d = nc.declare_dram_parameter("x_nat", [B, NIC, 4, 128, 2, 512], BF16,
                                     isOutput=False)
    eb_d = nc.declare_dram_parameter("ebT", [2, N, N], BF16, isOutput=False)
    wq_d = nc.declare_dram_parameter("wq", [DIM, EH], BF16, isOutput=False)
    wk_d = nc.declare_dram_parameter("wk", [DIM, EH], BF16, isOutput=False)
    wv_d = nc.declare_dram_parameter("wv", [DIM, EH], BF16, isOutput=False)
    wo_d = nc.declare_dram_parameter("wo", [EH, DIM], BF16, isOutput=False)
    qs2_d = nc.declare_dram_parameter("qs2", [EH], F32, isOutput=False)
    kb_d = nc.declare_dram_parameter("kb", [B, N], F32, isOutput=False)
    out_d = nc.declare_dram_parameter("out_part", [B, N, DIM], BF16,
                                      isOutput=True)

    with tile.TileContext(nc) as tc:
        import contextlib
        with contextlib.ExitStack() as ctx:
            pers = ctx.enter_context(tc.tile_pool(name="pers", bufs=1))
            # SBUF pools
            xrp = ctx.enter_context(tc.tile_pool(name="xrp", bufs=2))
            xnp = ctx.enter_context(tc.tile_pool(name="xnp", bufs=2))
            eb_pool = ctx.enter_context(tc.tile_pool(name="eb_pool", bufs=2))
            rowp = ctx.enter_context(tc.tile_pool(name="rowp", bufs=1))
            colp = ctx.enter_context(tc.tile_pool(name="colp", bufs=2))
            sqp = ctx.enter_context(tc.tile_pool(name="sqp", bufs=2))
            srp = ctx.enter_context(tc.tile_pool(name="srp", bufs=1))
            ep = ctx.enter_context(tc.tile_pool(name="ep", bufs=3))
            emp = ctx.enter_context(tc.tile_pool(name="emp", bufs=5))
            anp = ctx.enter_context(tc.tile_pool(name="anp", bufs=2))
            obp = ctx.enter_context(tc.tile_pool(name="obp", bufs=2))
            # PSUM pools: tl(4) + sp(2) + av(2) = 8 banks
            tl_ps = ctx.enter_context(tc.tile_pool(name="tl_ps", bufs=4, space="PSUM"))
            sp_ps = ctx.enter_context(tc.tile_pool(name="sp_ps", bufs=2, space="PSUM"))
            av_ps = ctx.enter_context(tc.tile_pool(name="av_ps", bufs=2, space="PSUM"))

            # ---------- constants ----------
            onescol_f = pers.tile([128, 1], F32, tag="onescol_f")
            nc.vector.memset(onescol_f, 1.0)
            onescol_bf = pers.tile([128, 1], BF16, tag="onescol_bf")
            nc.vector.tensor_copy(out=onescol_bf, in_=onescol_f)
            warm_row = pers.tile([1, 512], BF16, tag="warm_row")
            nc.vector.memset(warm_row, 1.0)
            # q-side block-ones with 1/64 folded (ssq = |q|^2/64 per head)
            o2q_f = pers.tile([128, 2], F32, tag="o2q_f")
            nc.vector.memset(o2q_f, 0.0)
            nc.vector.memset(o2q_f[0:64, 0:1], 1.0 / 64.0)
            nc.vector.memset(o2q_f[64:128, 1:2], 1.0 / 64.0)
            ones2blk_q = pers.tile([128, 2], BF16, tag="ones2blk_q")
            nc.vector.tensor_copy(out=ones2blk_q, in_=o2q_f)
            ident = pers.tile([128, 128], F32, tag="ident")
            make_identity(nc, ident)
            ident_bf = pers.tile([128, 128], BF16, tag="ident_bf")
            nc.vector.tensor_copy(out=ident_bf, in_=ident)

            # ---- PE warm-up: dummy matmuls while the first DMAs stream ----
            warm_ps = av_ps.tile([1, IC], F32, tag="av")
            for _ in range(N_WARM):
                nc.tensor.matmul(warm_ps, onescol_bf[0:1, :], warm_row,
                                 start=True, stop=True)

            # ---------- weights (gamma-folded + centered on host) ----------
            wps = {}
            for nm, wd in (("q", wq_d),):
                wp = pers.tile([128, NCT, EH], BF16, tag=f"w{nm}p")
                nc.sync.dma_start(out=wp, in_=wd.ap().rearrange("(t p) e -> p t e", p=128))
                wps[nm] = wp

            # scale rows -> block-diag [2, 128] (qsb[h, e] = s[e] iff head(e)==h)
            # where s = q_scale * k_scale / 8 (prepped on host)
            qsb_f = pers.tile([2, 128], F32, tag="qsb_f")
            nc.vector.memset(qsb_f, 0.0)
            nc.sync.dma_start(out=qsb_f[0:1, 0:64], in_=qs2_d.ap()[0:64].unsqueeze(0))
            nc.sync.dma_start(out=qsb_f[1:2, 64:128], in_=qs2_d.ap()[64:128].unsqueeze(0))
            qs2blk = pers.tile([2, 128], F32R, tag="qs2blk")
            nc.vector.tensor_copy(out=qs2blk, in_=qsb_f)

            kbT = pers.tile([128, B, NJT], F32, tag="kbT")
            nc.sync.dma_start(out=kbT, in_=kb_d.ap().rearrange("b (t p) -> p b t", p=128))

            wo_sb = pers.tile([128, DIM], BF16, tag="wo_sb")
            nc.sync.dma_start(out=wo_sb, in_=wo_d.ap())

            # ---------- persistent per-batch products ----------
            qhat = [pers.tile([128, N], BF16, tag=f"qhat{b}", name=f"qhat{b}") for b in range(B)]
            khat = [pers.tile([128, N], BF16, tag=f"khat{b}", name=f"khat{b}") for b in range(B)]
            v_all = [pers.tile([128, NJT, 130], BF16, tag=f"vall{b}", name=f"vall{b}") for b in range(B)]
            rnk = [pers.tile([128, NIC, 4, 2], F32, tag=f"rnk{b}", name=f"rnk{b}")
                   for b in range(B)]
            for b in range(B):
                # softmax-denominator columns (65h+64) are constant 1.0
                for jt in range(NJT):
                    nc.vector.memset(v_all[b][:, jt, 64:65], 1.0)
                    nc.vector.memset(v_all[b][:, jt, 129:130], 1.0)

            def emit_xr_chunk(icn):
                """xT chunk tiles [128, NCT, IC] per b, from a 2-buf ring."""
                xrs = []
                for b in range(B):
                    xr = xrp.tile([128, NCT, IC], BF16, tag=f"xr{b}",
                                  name=f"xr{b}_{icn}")
                    for half in range(2):
                        hs = slice(half * (NCT // 2), (half + 1) * (NCT // 2))
                        nc.sync.dma_start(
                            out=xr[:, hs, :],
                            in_=xT_d.ap()[b, icn, half].rearrange(
                                "(t p) n -> p t n", p=128))
                    xrs.append(xr)
                return xrs

            def emit_xna_chunk(ic):
                xnas = []
                for b in range(B):
                    xna = xnp.tile([128, 4, 2, 512], BF16, tag=f"xna{b}",
                                   name=f"xna{b}_{ic}")
                    nc.scalar.dma_start(
                        out=xna, in_=xn_d.ap()[b, ic].rearrange("t p g f -> p t g f"))
                    xnas.append(xna)
                return xnas

            def eb_prefetch(ic):
                """EB chunk tile [128, NJT, 2, IC] (jt-major, head-interleaved)."""
                isl = slice(ic * IC, (ic + 1) * IC)
                jmax = 4 * (ic + 1)
                ebt = eb_pool.tile([128, NJT, 2, IC], BF16, tag="eb",
                                   name=f"eb{ic}")
                for h in range(2):
                    nc.gpsimd.dma_start(
                        out=ebt[:, 0:jmax, h, :],
                        in_=eb_d.ap()[h, 0:jmax * 128, isl].rearrange(
                            "(t p) i -> p t i", p=128))
                return ebt

            def stats_units(ic, xnas, rstd_out):
                """LN variance per token (DVE bn_stats) -> rstd columns."""
                for b in range(B):
                    xna = xnas[b]
                    bag_all = colp.tile([128, 4, 2], F32, tag="bag")
                    for k in range(IC // 128):
                        bst = colp.tile([128, 2, 6], F32, tag="bst")
                        nc.vector.bn_stats(out=bst[:, 0, :], in_=xna[:, k, 0, :])
                        nc.vector.bn_stats(out=bst[:, 1, :], in_=xna[:, k, 1, :])
                        nc.vector.bn_aggr(out=bag_all[:, k, :], in_=bst)
                        yield
                    rstd_all = colp.tile([128, 4], F32, tag="rstdc")
                    _emit_rsqrt(nc, nc.vector, rstd_all, bag_all[:, :, 1], colp, "rsc")
                    rstd_out[b] = rstd_all
                    yield

            def phase1_units(ic, xrs, rstd_cols):
                """Chunk-ic projections + l2norm + v transpose, emitted
                batch-sequentially so phase2(ic, b0) can start as soon as
                batch 0 is done."""
                isl = slice(ic * IC, (ic + 1) * IC)
                for b in range(B):
                    xr = xrs[b]
                    # q projection + l2norm chain
                    p = tl_ps.tile([128, IC], F32, tag="tl", name=f"pq{b}")
                    for ct in range(NCT):
                        nc.tensor.matmul(p, wps["q"][:, ct, :], xr[:, ct, :],
                                         start=(ct == 0), stop=(ct == NCT - 1))
                    yield
                    sq = sqp.tile([128, IC], BF16, tag="sq", name=f"sqq{b}")
                    nc.scalar.activation(out=sq, in_=p, func=AF.Square)
                    ssq = sp_ps.tile([2, IC], F32, tag="sp", name=f"ssq{b}")
                    nc.tensor.matmul(ssq, ones2blk_q, sq, start=True, stop=True)
                    rnr = rowp.tile([2, IC], F32R, tag="rowtmp")
                    _emit_rsqrt(nc, nc.vector, rnr, ssq, rowp, "rsr")
                    sr = sp_ps.tile([128, IC], F32, tag="sp", name=f"sr{b}")
                    nc.tensor.matmul(sr, qs2blk, rnr, start=True, stop=True)
                    srs = srp.tile([128, IC], F32, tag="srs")
                    nc.vector.tensor_copy(out=srs, in_=sr)
                    nc.vector.tensor_mul(qhat[b][:, isl], p, srs)
                    yield
                    # k projection + rn_k columns
                    p = tl_ps.tile([128, IC], F32, tag="tl", name=f"pk{b}")
                    for ct in range(NCT):
                        nc.tensor.matmul(p, wps["k"][:, ct, :], xr[:, ct, :],
                                         start=(ct == 0), stop=(ct == NCT - 1))
                    yield
                    sq = sqp.tile([128, IC], BF16, tag="sq", name=f"sqk{b}")
                    nc.scalar.activation(out=sq, in_=p, func=AF.Square)
                    nc.scalar.copy(out=khat[b][:, isl], in_=p)
                    rncol = sp_ps.tile([128, 4, 2], F32, tag="sp", name=f"rnc{b}")
                    for jb in range(4):
                        nc.tensor.matmul(rncol[:, jb, :],
                                         sq[:, jb * 128:(jb + 1) * 128],
                                         ones2blk_q, start=True, stop=True)
                    _emit_rsqrt(nc, nc.vector, rnk[b][:, ic], rncol, colp, "rsk")
                    yield
                    # v projection + transpose + rstd row scale
                    p = tl_ps.tile([128, IC], F32, tag="tl", name=f"pv{b}")
                    for ct in range(NCT):
                        nc.tensor.matmul(p, wps["v"][:, ct, :], xr[:, ct, :],
                                         start=(ct == 0), stop=(ct == NCT - 1))
                    yield
                    rstd_all = rstd_cols[b]
                    vsc = sqp.tile([128, IC], BF16, tag="vsc")
                    nc.scalar.copy(out=vsc, in_=p)
                    for k in range(IC // 128):
                        jt = ic * (IC // 128) + k
                        vt = sp_ps.tile([128, 128], BF16, tag="sp", name=f"vt{b}{k}")
                        nc.tensor.transpose(vt, vsc[:, k * 128:(k + 1) * 128], ident_bf)
                        nc.vector.tensor_scalar_mul(
                            out=v_all[b][:, jt, 0:64], in0=vt[:, 0:64],
                            scalar1=rstd_all[:, k:k + 1])
                        nc.vector.tensor_scalar_mul(
                            out=v_all[b][:, jt, 65:129], in0=vt[:, 64:128],
                            scalar1=rstd_all[:, k:k + 1])
                    yield

            def normwo_units(ic, b, avs):
                """Normalize chunk-ic attention outputs and run the Wo partial
                matmuls.  avs = [h0, h1] PSUM tiles [65, IC]."""
                # 1/S rows: f32 reciprocal -> bf16 row for the ones-broadcast
                rec = rowp.tile([1, 2, IC], F32, tag="rec")
                nc.vector.tensor_copy(out=rec[:, 0, :], in_=avs[0][64:65, :])
                nc.vector.tensor_copy(out=rec[:, 1, :], in_=avs[1][64:65, :])
                nc.vector.reciprocal_approx_fast(out=rec, in_=rec)
                recb = rowp.tile([1, 2, IC], BF16, tag="recb")
                nc.vector.tensor_copy(out=recb, in_=rec)
                recbs = [recb[:, 0, :], recb[:, 1, :]]
                yield
                bcb = sp_ps.tile([128, IC], F32, tag="sp", name=f"bcb{ic}{b}")
                nc.tensor.matmul(bcb[0:64, :], warm_row[:, 0:64], recbs[0],
                                 start=True, stop=True)
                nc.tensor.matmul(bcb[64:128, :], warm_row[:, 0:64], recbs[1],
                                 start=True, stop=True)
                bcs = srp.tile([128, IC], F32, tag="bcs")
                nc.vector.tensor_copy(out=bcs, in_=bcb)
                yield
                an = anp.tile([128, IC], BF16, tag="an")
                nc.vector.tensor_mul(an[0:64, :], avs[0][0:64, :], bcs[0:64, :])
                nc.vector.tensor_mul(an[64:128, :], avs[1][0:64, :], bcs[64:128, :])
                yield
                for it in range(4):
                    ob = obp.tile([128, DIM], BF16, tag="ob")
                    for oc in range(2):
                        op = sp_ps.tile([128, 512], F32, tag="sp",
                                        name=f"op{ic}{b}{it}{oc}")
                        nc.tensor.matmul(op, an[:, it * 128:(it + 1) * 128],
                                         wo_sb[:, oc * 512:(oc + 1) * 512],
                                         start=True, stop=True)
                        if oc == 0:
                            nc.vector.tensor_copy(out=ob[:, 0:512], in_=op)
                        else:
                            nc.scalar.copy(out=ob[:, 512:1024], in_=op)
                    nc.sync.dma_start(
                        out=out_d.ap()[b, ic * IC + it * 128:ic * IC + (it + 1) * 128, :],
                        in_=ob)
                    yield

            def phase2_chunk(ic, ebt, filler, pull_k):
                jmax = (IC // 128) * (ic + 1)
                avs_last = None
                for b in range(B):
                    avs = [av_ps.tile([65, IC], F32, tag="av", name=f"av{ic}{b}{h}")
                           for h in range(2)]
                    tiles = []
                    for jt in range(jmax):
                        diag_k = jt - (jmax - 4)
                        off = 128 * diag_k if diag_k > 0 else 0
                        tiles.append((jt, off))
                    # groups of 2 jt (= 4 (jt,h) tiles): sim mms back-to-back,
                    # exps, EB muls, pull filler, then the PREVIOUS group's
                    # av mms (their inputs are certainly ready).
                    groups = [tiles[i:i + 2] for i in range(0, len(tiles), 2)]
                    pend_group = []
                    for g in groups:
                        cur = []
                        for (jt, off) in g:
                            icj, jb = jt // 4, jt % 4
                            mode = _eb_mode(jt)
                            E = ep.tile([128, 2, IC], BF16, tag="E")
                            for h in range(2):
                                dsl = slice(64 * h, 64 * h + 64)
                                sp = tl_ps.tile([128, IC], F32, tag="tl", name="sim")
                                if mode == 0:
                                    # PE identity-injects the raw bias; NOTE:
                                    # the injected bias is NOT scaled by rn_k,
                                    # so the host pre-divides... see _eb_mode.
                                    nc.tensor.matmul(
                                        sp[:, off:], ident_bf,
                                        ebt[:, jt, h, off:],
                                        start=True, stop=False)
                                    nc.tensor.matmul(
                                        sp[:, off:], khat[b][dsl, jt * 128:(jt + 1) * 128],
                                        qhat[b][dsl, ic * IC + off:(ic + 1) * IC],
                                        start=False, stop=True)
                                else:
                                    nc.tensor.matmul(
                                        sp[:, off:], khat[b][dsl, jt * 128:(jt + 1) * 128],
                                        qhat[b][dsl, ic * IC + off:(ic + 1) * IC],
                                        start=True, stop=True)
                                nc.scalar.activation(
                                    out=E[:, h, off:], in_=sp[:, off:], func=AF.Exp,
                                    bias=kbT[:, b, jt:jt + 1],
                                    scale=rnk[b][:, icj, jb, h:h + 1])
                            if mode == 0:
                                cur.append((E, jt, off))
                            else:
                                Em = emp.tile([128, 2, IC], BF16, tag="Em")
                                eng = nc.gpsimd if mode == 2 else nc.vector
                                eng.tensor_mul(Em[:, :, off:], E[:, :, off:],
                                               ebt[:, jt, :, off:])
                                cur.append((Em, jt, off))
                        filler.pull(pull_k)
                        for (pEm, pjt, poff) in pend_group:
                            for h in range(2):
                                nc.tensor.matmul(
                                    avs[h][:, poff:],
                                    v_all[b][:, pjt, 65 * h:65 * h + 65],
                                    pEm[:, h, poff:], start=(pjt == 0),
                                    stop=(pjt == jmax - 1))
                        pend_group = cur
                    for (pEm, pjt, poff) in pend_group:
                        for h in range(2):
                            nc.tensor.matmul(
                                avs[h][:, poff:], v_all[b][:, pjt, 65 * h:65 * h + 65],
                                pEm[:, h, poff:], start=(pjt == 0),
                                stop=(pjt == jmax - 1))
                    if b == 0:
                        # b0's norm+Wo becomes the front of the filler so its
                        # avs PSUM bank frees before b1's av accumulation.
                        filler.prepend(normwo_units(ic, 0, avs))
                        filler.pull(3)
                    else:
                        avs_last = avs
                return avs_last

            # ---------------- schedule ----------------
            xrs0 = emit_xr_chunk(0)
            ebt = eb_prefetch(0)
            for nm, wd in (("k", wk_d), ("v", wv_d)):
                wp = pers.tile([128, NCT, EH], BF16, tag=f"w{nm}p")
                nc.sync.dma_start(out=wp, in_=wd.ap().rearrange("(t p) e -> p t e", p=128))
                wps[nm] = wp
            xnas0 = emit_xna_chunk(0)
            rstd0 = {}
            stats0 = stats_units(0, xnas0, rstd0)
            p1_0 = phase1_units(0, xrs0, rstd0)
            # batch-0 of phase1(0) (+ its stats before the v epilogue); batch 1
            # is pulled into phase2(0, b0) as filler
            for _ in range(5):
                next(p1_0, None)
            for _ in range(5):
                next(stats0, None)
            next(p1_0, None)
            for _ in stats0:
                pass
            for _ in p1_0:
                pass

            xrs_next = emit_xr_chunk(1)
            xnas_next = emit_xna_chunk(1)

            pend_b1 = None   # (ic, avs) of previous chunk's b1 awaiting norm+Wo
            for ic in range(NIC):
                nebt = eb_prefetch(ic + 1) if ic + 1 < NIC else None
                fillers = []
                if ic == 0:
                    fillers.append(stats0)
                    fillers.append(p1_0)
                if pend_b1 is not None:
                    fillers.append(normwo_units(pend_b1[0], 1, pend_b1[1]))
                nrstd = {}
                if ic + 1 < NIC:
                    fillers.append(stats_units(ic + 1, xnas_next, nrstd))
                    fillers.append(phase1_units(ic + 1, xrs_next, nrstd))
                filler = _Chain(*fillers)
                groups_per_b = max(1, (4 * (ic + 1)) // 2)
                pull_k = max(2, min(6, -(-36 // (2 * groups_per_b))))
                avs_b1 = phase2_chunk(ic, ebt, filler, pull_k)
                filler.drain()
                pend_b1 = (ic, avs_b1)
                if ic + 2 < NIC:
                    xrs_next = emit_xr_chunk(ic + 2)
                    xnas_next = emit_xna_chunk(ic + 2)
                ebt = nebt
            # tail: last chunk's b1 norm+Wo
            for _ in normwo_units(pend_b1[0], 1, pend_b1[1]):
                pass
    nc.compile()
    return nc


PROFILE = {"enabled": False, "a_ns": None, "b_ns": None}


def _install_profile_hook():
    """Register the axon NTFF profile hook (the image's antenv lacks
    axon_hooks, so run_bass_kernel_spmd(trace=True) would silently skip
    tracing).  Replicates trn_boot's ctypes recipe."""
    import sys, types, ctypes, contextlib

    if "antenv.axon_hooks" in sys.modules:
        return
    lib = ctypes.CDLL("/opt/axon/libaxon_pjrt.so")
    if not hasattr(lib, "axon_start_nrt_profile"):
        return
    lib.axon_start_nrt_profile.argtypes = [ctypes.POINTER(ctypes.c_int64), ctypes.c_size_t]
    lib.axon_start_nrt_profile.restype = ctypes.c_int64
    lib.axon_stop_nrt_profile.argtypes = [ctypes.c_char_p]
    lib.axon_stop_nrt_profile.restype = ctypes.c_int64

    @contextlib.contextmanager
    def _hook(output_dir, device_ids):
        import jax
        jax.devices()
        if device_ids:
            ids = (ctypes.c_int64 * len(device_ids))(*device_ids)
            rc = lib.axon_start_nrt_profile(ids, len(device_ids))
        else:
            rc = lib.axon_start_nrt_profile(None, 0)
        if rc != 0:
            raise RuntimeError(f"axon_start_nrt_profile rc={rc}")
        try:
            yield
        finally:
            n = lib.axon_stop_nrt_profile(str(output_dir).encode())
            print(f"profile: {n} file(s) written to {output_dir}")

    mod = types.ModuleType("antenv.axon_hooks")
    mod.get_axon_ntff_profile_hook = lambda: _hook
    mod.set_axon_ntff_profile_hook = lambda h: None
    sys.modules["antenv.axon_hooks"] = mod

    from concourse import bass_utils
    bass_utils.upload_artifacts = lambda tmpdir: ""


def kernel(x, gamma, Wq, Wkv, q_scale, k_scale, Wo, rel_pos_bias, mask):
    from concourse.bass_utils import run_bass_kernel_spmd
    import ml_dtypes

    BF = ml_dtypes.bfloat16
    x = np.ascontiguousarray(np.asarray(x, dtype=np.float32))
    gamma = np.asarray(gamma, dtype=np.float32)
    Wq = np.asarray(Wq, dtype=np.float32)
    Wkv = np.asarray(Wkv, dtype=np.float32)
    q_scale = np.asarray(q_scale, dtype=np.float32)
    k_scale = np.asarray(k_scale, dtype=np.float32)
    Wo = np.ascontiguousarray(np.asarray(Wo, dtype=np.float32))
    rel_pos_bias = np.asarray(rel_pos_bias, dtype=np.float32)
    mask = np.asarray(mask)

    if PROFILE["enabled"]:
        _install_profile_hook()
    if "a" not in _cache:
        _cache["a"] = _build_launch()

    # xT layout: [B, NIC, half, 512(dim-part), IC]
    xT = np.ascontiguousarray(x.transpose(0, 2, 1)).astype(BF)   # [B, DIM, N]
    xT = np.ascontiguousarray(
        xT.reshape(B, 2, 512, NIC, IC).transpose(0, 3, 1, 2, 4))
    x_nat = np.ascontiguousarray(x.reshape(B, NIC, 4, 128, 2, 512)).astype(BF)
    kb = np.where(mask, 0.0, NEG).astype(np.float32)
    # combined q-side scale: q_scale * k_scale / 8  (k-side l2norm rides the
    # Exp scale operand as rn_k = 8/|k|; (1/8)*8*8 = SCALE=8 total)
    qs2 = np.tile(q_scale * k_scale / 8.0, 2).astype(np.float32)

    # gamma-fold + column-center the projection weights (exact LN mean-sub)
    def prep_w(W):
        Wg = W * gamma[:, None]
        return (Wg - Wg.mean(axis=0, keepdims=True)).astype(BF)

    Wq_p = prep_w(Wq)
    Wk_p = prep_w(Wkv[:, :DIM])
    Wv_p = prep_w(Wkv[:, DIM:])

    # rel_pos bias: transpose to [h, j, i]; per j-tile either RAW masked bias
    # (PE-injected pre-exp) or exp(bias) masked to 0 (post-exp multiply)
    rpbT = np.ascontiguousarray(rel_pos_bias.transpose(0, 2, 1))
    tri = np.tril(np.ones((N, N), dtype=bool), -1)   # [j, i]: j > i masked
    EB = np.exp(rpbT, dtype=np.float32)
    EB[:, tri] = 0.0
    EB = EB.astype(BF)

    Wo_bf = Wo.astype(BF)
    in_maps = []
    for c in range(NCORES):
        es = slice(EH * c, EH * (c + 1))
        in_maps.append({
            "xT": xT,
            "x_nat": x_nat,
            "ebT": np.ascontiguousarray(EB[2 * c:2 * c + 2]),
            "wq": np.ascontiguousarray(Wq_p[:, es]),
            "wk": np.ascontiguousarray(Wk_p[:, es]),
            "wv": np.ascontiguousarray(Wv_p[:, es]),
            "wo": np.ascontiguousarray(Wo_bf[es, :]),
            "qs2": qs2, "kb": kb,
        })
    res = run_bass_kernel_spmd(_cache["a"], in_maps, list(range(NCORES)),
                               trace=PROFILE["enabled"])
    if PROFILE["enabled"]:
        PROFILE["a_ns"] = res.exec_time_ns
        PROFILE["b_ns"] = 0

    out = np.zeros((B, N, DIM), np.float32)
    for c in range(NCORES):
        out += res.results[c]["out_part"].astype(np.float32)
    return out
